# revision 14
# baseline (speedup 1.0000x reference)
"""Boolean OR-matmul kernel for Trainium2 (8 NeuronCores).

out[b, i] = OR_j (x[b, j] AND w[i, j])  ==  (x_f32 @ w.T_f32) > 0

Strategy:
- Shard bit_weights rows (layer_size 8192) across 8 cores -> 1024 rows/core,
  replicate x. No cross-core reduction needed; host concatenates column
  blocks of the output.
- Encode bools as fp8_e4m3 0.0/1.0 (bit pattern 0x38 == 1.0). Products are
  exactly 0/1, PSUM accumulates fp32 (counts <= 8192 < 2^24, exact), so
  (count > 0) is exact.
- Host pre-tiles both operands into the exact (partition, k-subtile, col)
  blocks the kernel consumes, so every DMA is a single fully-contiguous
  32-64KB run per partition (large descriptors -> ~2x the HBM bandwidth of
  the 0.5-1KB strided runs a (D, B) layout produces).
- PE does fp8 DoubleRow matmuls (K=256 per instruction), k-innermost per
  PSUM tile so the accumulation group stays dense.
- A short chain of dummy matmuls on a zeroed tile warms the PE HAM clock
  gate during the DMA preload, so real matmuls start at 2.4 GHz.
- DVE thresholds PSUM fp32 -> uint8 0/1 via is_gt; output DMAs go out on
  the scalar-engine HWDGE ring to stay off the input ring.
"""

import sys

for _p in ("/opt/trn_rl_repo",):
    if _p not in sys.path:
        sys.path.insert(0, _p)

import numpy as np
import ml_dtypes

import concourse.bass as bass
import concourse.tile as tile
from concourse import bacc, mybir
from concourse.bass_utils import run_bass_kernel_spmd

P = 128          # SBUF partitions / PE contraction per k-subtile
N_CORES = 8

# Full problem shapes (hardcoded per harness contract)
BATCH = 4096
IN_DIM = 8192
LAYER_SIZE = 8192
L_SHARD = LAYER_SIZE // N_CORES  # 1024

N_WARM = 8       # dummy matmuls to warm the PE clock gate during preload


def build_nc(B, D, L, b_slab=512, n_free=512):
    """Build the per-core Bass program.

    Per-core inputs : xT4 (P, NS, KSUB, b_slab) fp8e4  [host pre-tiled]
                      wT3 (P, KSUB, L) fp8e4           [host pre-tiled]
    Per-core output : out (B, L) uint8 (0/1)
    """
    assert D % (2 * P) == 0 and B % b_slab == 0 and b_slab % P == 0
    assert L % n_free == 0
    KSUB = D // P               # k-subtiles of 128
    NL = L // n_free            # l tiles
    NS = B // b_slab            # slabs
    MSUB = b_slab // P

    nc = bacc.Bacc(None, target_bir_lowering=False, debug=False)
    xT4 = nc.dram_tensor(
        "xT4", [P, NS, KSUB, b_slab], mybir.dt.float8e4, kind="ExternalInput"
    )
    wT3 = nc.dram_tensor("wT3", [P, KSUB, L], mybir.dt.float8e4, kind="ExternalInput")
    out = nc.dram_tensor("out", [B, L], mybir.dt.uint8, kind="ExternalOutput")

    # k-chunk boundaries (in k-subtiles) for the slab-0 / weight preload.
    # Graduated sizes: small leading chunks gate the first matmul on ~300KB
    # of DMA instead of 12MB; larger trailing chunks keep descriptor and
    # Sync-queue issue counts low. Boundaries stay even so DoubleRow k-pairs
    # never straddle a chunk.
    bounds = sorted({b for b in (0, 2, 4, 8, 16, 32) if b < KSUB} | {KSUB})
    chunks = list(zip(bounds[:-1], bounds[1:]))  # [(lo, hi), ...]
    ks2chunk = {}
    for ci, (lo, hi) in enumerate(chunks):
        for ks in range(lo, hi):
            ks2chunk[ks] = (ci, ks - lo)

    with tile.TileContext(nc) as tc:
        with (
            tc.tile_pool(name="wpool", bufs=1) as wpool,
            tc.tile_pool(name="xpool", bufs=1) as xpool,
            tc.tile_pool(name="opool", bufs=4) as opool,
            tc.tile_pool(name="psum", bufs=8, space="PSUM") as pspool,
        ):
            # --- PE warm-up: keep the HAM activity monitor busy during the
            # DMA preload so the first real matmuls run at 2.4 GHz. The
            # chain accumulates into a scratch PSUM tile that is never read.
            # The operand memset rides the otherwise-idle GpSimd queue so
            # the chain starts as soon as the engines clear the preamble
            # barrier (tile values are irrelevant, but Tile requires a
            # writer before a read).
            warm = wpool.tile([P, n_free], mybir.dt.float8e4, tag="warm", name="warm")
            nc.gpsimd.memset(warm[:], 0)
            wps = pspool.tile([P, n_free], mybir.dt.float32, tag="ps", name="wps")
            for i in range(N_WARM):
                nc.tensor.matmul(
                    wps[:],
                    warm[:, 0:P],
                    warm[:],
                    start=(i == 0),
                    stop=(i == N_WARM - 1),
                    skip_group_check=True,
                )

            w_tiles = [
                wpool.tile(
                    [P, hi - lo, L], mybir.dt.float8e4, tag=f"w{j}", name=f"w{j}"
                )
                for j, (lo, hi) in enumerate(chunks)
            ]
            # Slab-0 x arrives in the same k-chunks, interleaved with the
            # weight loads in k-consumption order (x before w: the x tile
            # gates LDWEIGHTS, w gates the matmul). Everything stays on the
            # ONE Sync HWDGE ring, in consumption order: the SDMA engines
            # round-robin between rings at packet granularity, so any
            # concurrent ring with bigger packets would starve the small
            # leading chunks exactly when they gate the first matmuls.
            x0_tiles = []
            for j, (lo, hi) in enumerate(chunks):
                xt = xpool.tile(
                    [P, hi - lo, b_slab], mybir.dt.float8e4,
                    tag=f"xc{j}", name=f"xc{j}",
                )
                nc.sync.dma_start(out=xt[:], in_=xT4[:, 0, lo:hi, :])
                nc.sync.dma_start(out=w_tiles[j][:], in_=wT3[:, lo:hi, :])
                x0_tiles.append(xt)

            def drain(ps, b0, m, l):
                # Output stores ride the Scalar HWDGE ring so they never
                # queue behind the input stream.
                ob = opool.tile([P, n_free], mybir.dt.uint8, tag="ob", name="ob")
                nc.vector.tensor_scalar(
                    out=ob[:],
                    in0=ps[:],
                    scalar1=0.0,
                    scalar2=None,
                    op0=mybir.AluOpType.is_gt,
                )
                nc.scalar.dma_start(
                    out=out[b0 + m * P : b0 + (m + 1) * P,
                            l * n_free : (l + 1) * n_free],
                    in_=ob[:],
                )

            groups = [(m, l) for m in range(MSUB) for l in range(NL)]

            # --- Slab 0: DMA-paced (the w+x preload is in flight). Run k
            # OUTERMOST across all groups, one PSUM bank each, so every
            # arriving k-chunk feeds MSUB*NL matmuls and the PE never
            # outruns the DMA wave.
            assert len(groups) <= 8
            pss = {
                g: pspool.tile([P, n_free], mybir.dt.float32, tag="ps", name="ps")
                for g in groups
            }
            for ks in range(0, KSUB, 2):
                ci, off = ks2chunk[ks]
                xt, wt = x0_tiles[ci], w_tiles[ci]
                for m, l in groups:
                    nc.tensor.matmul(
                        pss[(m, l)][:],
                        xt[:, off : off + 2, m * P : (m + 1) * P],
                        wt[:, off : off + 2, l * n_free : (l + 1) * n_free],
                        start=(ks == 0),
                        stop=(ks == KSUB - 2),
                        perf_mode=mybir.MatmulPerfMode.DoubleRow,
                        skip_group_check=True,
                    )
            for m, l in groups:
                drain(pss[(m, l)], 0, m, l)

            # --- Slabs 1..NS-1: one contiguous DMA per slab (double
            # buffered), k-innermost per PSUM group.
            for s in range(1, NS):
                xt = xpool.tile(
                    [P, KSUB, b_slab], mybir.dt.float8e4,
                    tag="x", bufs=2, name="x",
                )
                nc.sync.dma_start(out=xt[:], in_=xT4[:, s, :, :])
                b0 = s * b_slab
                for gi, (m, l) in enumerate(groups):
                    last = s == NS - 1 and gi == len(groups) - 1
                    if not last:
                        ps = pspool.tile(
                            [P, n_free], mybir.dt.float32, tag="ps", name="ps"
                        )
                        for ks in range(0, KSUB, 2):
                            ci, off = ks2chunk[ks]
                            nc.tensor.matmul(
                                ps[:],
                                xt[:, ks : ks + 2, m * P : (m + 1) * P],
                                w_tiles[ci][:, off : off + 2,
                                            l * n_free : (l + 1) * n_free],
                                start=(ks == 0),
                                stop=(ks == KSUB - 2),
                                perf_mode=mybir.MatmulPerfMode.DoubleRow,
                                skip_group_check=True,
                            )
                        drain(ps, b0, m, l)
                        continue
                    # The very last group runs as two half-width (N=256)
                    # accumulation groups so the exposed tail after the
                    # final matmul is a half-sized threshold+store, and the
                    # first half's drain overlaps the second half's stream.
                    half = n_free // 2
                    psh = [
                        pspool.tile([P, half], mybir.dt.float32,
                                    tag="ps", name="ps")
                        for _ in range(2)
                    ]
                    for ks in range(0, KSUB, 2):
                        ci, off = ks2chunk[ks]
                        for h in range(2):
                            nc.tensor.matmul(
                                psh[h][:],
                                xt[:, ks : ks + 2, m * P : (m + 1) * P],
                                w_tiles[ci][:, off : off + 2,
                                            l * n_free + h * half :
                                            l * n_free + (h + 1) * half],
                                start=(ks == 0),
                                stop=(ks == KSUB - 2),
                                perf_mode=mybir.MatmulPerfMode.DoubleRow,
                                skip_group_check=True,
                            )
                    for h in range(2):
                        ob = opool.tile([P, half], mybir.dt.uint8,
                                        tag="ob", name="ob")
                        nc.vector.tensor_scalar(
                            out=ob[:],
                            in0=psh[h][:],
                            scalar1=0.0,
                            scalar2=None,
                            op0=mybir.AluOpType.is_gt,
                        )
                        nc.scalar.dma_start(
                            out=out[b0 + m * P : b0 + (m + 1) * P,
                                    l * n_free + h * half :
                                    l * n_free + (h + 1) * half],
                            in_=ob[:],
                        )
    nc.compile()
    return nc


def _to_fp8(u8_arr):
    """uint8 0/1 array -> fp8_e4m3 bytes holding 0.0 / 1.0 (0x38)."""
    return (np.ascontiguousarray(u8_arr) * np.uint8(0x38)).view(
        ml_dtypes.float8_e4m3
    )


def pack_x(x_u8, B, D, b_slab=512):
    """(B, D) 0/1 -> (P, NS, KSUB, b_slab) fp8, contiguous per partition."""
    NS, KSUB = B // b_slab, D // P
    xt = x_u8.reshape(NS, b_slab, KSUB, P).transpose(3, 0, 2, 1)
    return _to_fp8(xt)


def pack_w(w_u8, D, L):
    """(L, D) 0/1 -> (P, KSUB, L) fp8, contiguous per partition."""
    KSUB = D // P
    wt = w_u8.reshape(L, KSUB, P).transpose(2, 1, 0)
    return _to_fp8(wt)


_NC_CACHE = {}


def _get_nc(B, D, L, b_slab):
    key = (B, D, L, b_slab)
    if key not in _NC_CACHE:
        _NC_CACHE[key] = build_nc(B, D, L, b_slab=b_slab)
    return _NC_CACHE[key]


def run_spmd(x, bit_weights, trace=False, B=BATCH, D=IN_DIM, L_total=LAYER_SIZE,
             b_slab=512):
    """Shared runner: returns (full bool output, BassKernelResults)."""
    n = N_CORES
    L = L_total // n
    nc = _get_nc(B, D, L, b_slab)

    xT4 = pack_x(x.view(np.uint8), B, D, b_slab)
    w_u8 = bit_weights.view(np.uint8)
    in_maps = []
    for m in range(n):
        wT3 = pack_w(w_u8[m * L : (m + 1) * L, :], D, L)
        in_maps.append({"xT4": xT4, "wT3": wT3})

    res = run_bass_kernel_spmd(nc, in_maps, core_ids=list(range(n)), trace=trace)
    full = np.concatenate([res.results[m]["out"] for m in range(n)], axis=1)
    return full.view(np.bool_), res


def kernel(x, bit_weights):
    full, _ = run_spmd(np.asarray(x), np.asarray(bit_weights))
    return full
